# revision 35
# baseline (speedup 1.0000x reference)
"""Causal self-attention (B=2, T=2048, D=1024, 16 heads) on 8 trn2 cores.

Sharding: data-parallel over batch (4 cores per batch element), tensor-parallel
over heads (4 heads per core). Each core computes qkv/attention/proj for its
4 heads and produces a partial [T, D] projection output; the host sums the 4
partials of each batch element.

Schedule: the kernel is a software pipeline paced around the Activation
engine (exp is the second-largest engine load after the PE matmuls). Heads
rotate at (head, tq-slab) granularity: S^T strips stream onto PE and exp onto
ACT, while qkv projection / v / PV / output-projection matmuls are emitted as
PE filler between strips so the PE never waits on ACT's exp backlog. Input and
output DMAs are issued from SP; in the For_i timing build, UNROLL bodies are
emitted per loop iteration (For_i has an all-engine barrier per iteration)
with the output DMA software-pipelined one body behind its compute.
"""

from collections import deque

import numpy as np
import ml_dtypes

import concourse.mybir as mybir
import concourse.tile as tile
from concourse import bacc
from concourse.bass_utils import run_bass_kernel_spmd
from concourse.masks import make_identity, make_upper_triangular

B, T, D = 2, 2048, 1024
NH, DH = 16, 64
HPC = 4  # heads per core
NCORES = 8
KT = D // 128  # 8 contraction chunks for qkv matmuls
NT = T // 128  # 16 sequence chunks
SLAB = 1024  # tq columns per attention slab
NS = T // SLAB
import os
UNROLL = int(os.environ.get("K_UNROLL", "2"))  # bodies per For_i iteration

BF16 = mybir.dt.bfloat16
F16 = mybir.dt.float16
F32 = mybir.dt.float32
EXP = mybir.ActivationFunctionType.Exp

PE_NS = 1.0 / 2.4  # ns per PE cycle (full pstate)
ACT_NS = 1.0 / 1.2

TRACE = False
LAST_RESULTS = None
_NC_CACHE = {}


def _patch_ldw_opt():
    # walrus ships with the ldweights-optimization pass disabled; flipping it
    # lets codegen overlap/dedup PE weight loads (measured ~40-50ns each,
    # serialized, across ~1k matmuls here)
    import concourse.bass_utils as _bu

    if getattr(_bu, "_ldw_patched", False):
        return
    _orig = _bu.run_command

    def _rc(cmd, *a, **k):
        if isinstance(cmd, list):
            cmd = [
                "--enable-ldw-opt=true" if c == "--enable-ldw-opt=false" else c
                for c in cmd
            ]
        return _orig(cmd, *a, **k)

    _bu.run_command = _rc
    _bu._ldw_patched = True


if os.environ.get("K_LDWOPT"):
    _patch_ldw_opt()


def _build_program(loop_n=None, unroll=None, phase=5):
    nc = bacc.Bacc("TRN2", target_bir_lowering=False, debug=False, num_devices=NCORES)
    xT_d = nc.dram_tensor("xT", [D, T], BF16, kind="ExternalInput").ap()
    # wqk columns reordered host-side to [q01, k01, q23, k23] (128 each)
    wqk_d = nc.dram_tensor("wqk", [D, 2 * HPC * DH], BF16, kind="ExternalInput").ap()
    wv_d = nc.dram_tensor("wv", [D, HPC * DH], BF16, kind="ExternalInput").ap()
    wpr_d = nc.dram_tensor("wpr", [HPC * DH, D], BF16, kind="ExternalInput").ap()
    out_d = nc.dram_tensor("out", [T, D], F16, kind="ExternalOutput").ap()

    with tile.TileContext(nc) as tc:
        with (
            tc.tile_pool(name="setup", bufs=1) as setup,
            tc.tile_pool(name="big", bufs=1) as big,
            tc.tile_pool(name="pt_pool", bufs=2) as pt_pool,
            tc.tile_pool(name="stage", bufs=2) as stage,
            tc.tile_pool(name="ps_mm", bufs=int(os.environ.get("K_MMBUFS", "3")), space="PSUM") as ps_mm,
            tc.tile_pool(name="ps_s", bufs=int(os.environ.get("K_SBUFS", "3")), space="PSUM") as ps_s,
            tc.tile_pool(name="ps_y", bufs=2, space="PSUM") as ps_y,
        ):
            # ---- one-time setup (outside the For_i loop) ----
            ident = setup.tile([128, 128], BF16)
            make_identity(nc, ident)
            # exp table load happens here, off the steady-state critical path
            warm = setup.tile([128, 1], F32)
            nc.vector.memset(warm, 0.0)
            nc.scalar.activation(warm, warm, EXP)
            # gemask[p, f] = 1.0 where f >= p: valid (tq >= tk) part of a
            # diagonal 128x128 block of S^T.
            gemask = setup.tile([128, 128], BF16)
            make_upper_triangular(nc, gemask, val=1.0, diag=True)
            # v in natural [tk, d] layout + a ones-column per head for rowsums;
            # only the ones columns need initialization (col 64 of each 66).
            v_aug = setup.tile([128, NT, 66 * HPC], BF16)
            # persistent output staging: written by each body's copies, read
            # by the next body's (or the drain's) output DMAs
            ost = setup.tile([128, NT, D], F16)
            ones_cols = v_aug.rearrange("p n (h c) -> p n h c", c=66)[
                :, :, :, DH : DH + 1
            ]
            nc.vector.memset(ones_cols, 1.0)

            pools = (big, pt_pool, stage, ps_mm, ps_s, ps_y)
            consts = (ident, gemask, v_aug, ost)
            if unroll is not None:
                # straight-line repetition of the loop body for TimelineSim
                # steady-state analysis (For_i register loops aren't simulable)
                for _ in range(unroll):
                    _emit(nc, tc, pools, consts, xT_d, wqk_d, wv_d,
                          wpr_d, out_d, pipelined_out=(phase >= 5),
                          phase=phase)
                if phase >= 5:
                    _emit_out_dmas(nc, out_d, ost)
            elif loop_n is None:
                _emit(nc, tc, pools, consts, xT_d, wqk_d, wv_d, wpr_d, out_d,
                      pipelined_out=False)
            else:
                # For_i inserts an all-engine barrier per iteration, which
                # serializes consecutive bodies. Unroll UNROLL bodies per
                # iteration so they overlap (input DMAs / output DMAs / tail
                # of one body run under compute of the next).
                assert loop_n % UNROLL == 0, (loop_n, UNROLL)
                hints = (
                    mybir.EngineType.PE,
                    mybir.EngineType.Activation,
                    mybir.EngineType.DVE,
                    mybir.EngineType.SP,
                    mybir.EngineType.Pool,
                )
                with tc.For_i(0, loop_n // UNROLL, 1, hint_engines=hints,
                              staggered_reset=bool(os.environ.get("K_STAGGER"))):
                    for _ in range(UNROLL):
                        _emit(nc, tc, pools, consts, xT_d, wqk_d, wv_d,
                              wpr_d, out_d, pipelined_out=(phase >= 5),
                              phase=phase)
                # drain: the loop body DMAs the *previous* iteration's output,
                # so the final iteration's staged output goes out here
                if phase >= 5:
                    _emit_out_dmas(nc, out_d, ost)
    nc.compile()
    return nc


def _emit_out_dmas(nc, out_d, ost):
    for g in range(4):
        nc.sync.dma_start(
            out=out_d[512 * g : 512 * (g + 1), :].rearrange(
                "(j p) c -> p j c", p=128
            ),
            in_=ost[:, 4 * g : 4 * (g + 1), :],
        )


def _emit(nc, tc, pools, consts, xT_d, wqk_d, wv_d, wpr_d, out_d,
          pipelined_out=False, phase=5):
    # phase: 1=DMA only, 2=+qk/v, 3=+S/exp/mask, 4=+PV, 5=full
    big, pt_pool, stage, ps_mm, ps_s, ps_y = pools
    ident, gemask, v_aug, ost = consts

    xT_s = big.tile([128, KT, T], BF16, tag="xT")
    wqk_s = big.tile([128, KT, 2 * HPC * DH], BF16, tag="wqk")
    wv_s = big.tile([128, KT, HPC * DH], BF16, tag="wv")
    wpr_s = big.tile([128, 2, D], BF16, tag="wpr")
    # q^T / k^T in [d, T] layout: jt holds heads 2*jt (parts 0:64) and
    # 2*jt+1 (parts 64:128).
    qT_s = big.tile([128, 2, T], BF16, tag="qT")
    kT_s = big.tile([128, 2, T], BF16, tag="kT")
    y_all = big.tile([128, NT, HPC * DH], BF16, tag="y")
    yT_s = big.tile([128, 2, T], BF16, tag="yT")

    # ---- input DMAs, SP only; ordered for earliest first matmul ----
    xT_r = xT_d.rearrange("(a p) t -> p a t", p=128)
    wqk_r = wqk_d.rearrange("(a p) n -> p a n", p=128)
    nc.sync.dma_start(out=wqk_s[:, :, 0:256], in_=wqk_r[:, :, 0:256])
    nc.sync.dma_start(out=xT_s[:, :, 0:512], in_=xT_r[:, :, 0:512])
    nc.sync.dma_start(out=xT_s[:, :, 512:1024], in_=xT_r[:, :, 512:1024])
    nc.sync.dma_start(out=wqk_s[:, :, 256:512], in_=wqk_r[:, :, 256:512])
    nc.sync.dma_start(out=xT_s[:, :, 1024:1536], in_=xT_r[:, :, 1024:1536])
    nc.sync.dma_start(out=xT_s[:, :, 1536:2048], in_=xT_r[:, :, 1536:2048])
    nc.sync.dma_start(out=wv_s, in_=wv_d.rearrange("(a p) n -> p a n", p=128))
    nc.sync.dma_start(out=wpr_s, in_=wpr_d.rearrange("(a p) n -> p a n", p=128))

    if pipelined_out:
        # software-pipelined output: DMA the PREVIOUS body's staged output
        # (its copies finished by the end of that body), issued from ACT
        # after this body's input DMAs so inputs win the DMA engines first.
        # Body 0 ships garbage that later bodies' DMAs overwrite; the
        # post-loop drain ships the final body's data.
        _emit_out_dmas(nc, out_d, ost)
    # virtual completion estimates (ns) for pacing only
    rdy_wqkA, rdy_cb = 1500.0, [4400.0, 7300.0, 11700.0, 14600.0]
    rdy_wqkB, rdy_wv, rdy_wpr = 8800.0, 16100.0, 17700.0
    if phase == 1:
        return

    # ---- emission state ----
    st = {"pe": 0.0, "act": 0.0}
    qk_done = [[False] * 4 for _ in range(4)]  # [m'][n]; m' 0=q01 1=k01 2=q23 3=k23
    v_done = [False] * NT
    pv_done = [[False] * (SLAB // 128) for _ in range(2 * HPC)]  # [slab*4+h][jl]
    tp_done = [False] * NT

    def pe(cost, ready=0.0):
        st["pe"] = max(st["pe"], ready) + cost

    def emit_qk(m, n):
        # q^T or k^T block: wqk m'-block, T cols [512n, 512n+512)
        if qk_done[m][n]:
            return
        qk_done[m][n] = True
        pe(1707.0, max(rdy_wqkA if m < 2 else rdy_wqkB, rdy_cb[n]))
        ps = ps_mm.tile([128, 512], F32, tag="mm")
        for t in range(KT):
            nc.tensor.matmul(
                ps,
                lhsT=wqk_s[:, t, 128 * m : 128 * (m + 1)],
                rhs=xT_s[:, t, 512 * n : 512 * (n + 1)],
                start=(t == 0),
                stop=(t == KT - 1),
            )
        dst = qT_s if m % 2 == 0 else kT_s
        nc.vector.tensor_copy(dst[:, m // 2, 512 * n : 512 * (n + 1)], ps)

    def emit_v(j):
        # v rows [128j, 128j+128) = x @ wv, scattered into v_aug
        if v_done[j]:
            return
        v_done[j] = True
        pe(853.0, max(rdy_wv, rdy_cb[j // 4]))
        ps = ps_mm.tile([128, HPC * DH], F32, tag="mm")
        for t in range(KT):
            nc.tensor.matmul(
                ps,
                lhsT=xT_s[:, t, 128 * j : 128 * (j + 1)],
                rhs=wv_s[:, t, :],
                start=(t == 0),
                stop=(t == KT - 1),
            )
        nc.vector.tensor_copy(
            v_aug[:, j, :].rearrange("p (h c) -> p h c", c=66)[:, :, 0:DH],
            ps.rearrange("p (h c) -> p h c", c=DH),
        )

    def s_strip(h, s, i, pt, split=False):
        # S^T strip: tk block i vs tq cols [c_lo, SLAB*(s+1)) of slab s,
        # exp'd into pt[:, i, :] per <=512-col chunk. Each chunk gets its own
        # one-bank PSUM tile so the ps_s ring is 3 deep (more exp backlog
        # allowed before the PE stalls). Diagonal block gets gemask on DVE.
        jt, base = h // 2, 64 * (h % 2)
        qm, km = 2 * jt, 2 * jt + 1
        emit_qk(qm, 2 * s)
        if not split:
            emit_qk(qm, 2 * s + 1)
        emit_qk(km, i // 4)
        qT_h = qT_s[base : base + 64, jt, :]
        kT_h = kT_s[base : base + 64, jt, :]
        c_lo = max(SLAB * s, 128 * i)
        w = SLAB * (s + 1) - c_lo
        off = c_lo - SLAB * s
        # chunk at global 512-col boundaries (bank alignment)
        chunks, b = [], 0
        while b < w:
            e = min(w, b + 512 - (c_lo + b) % 512)
            chunks.append((b, e))
            b = e
        for ci, (c0, c1) in enumerate(chunks):
            if split and ci > 0:
                emit_qk(qm, (c_lo + c0) // 512)
            a0 = (c_lo + c0) % 512
            ps = ps_s.tile([128, 512], F32, tag="s")
            pe((c1 - c0) * PE_NS)
            nc.tensor.matmul(
                ps[:, a0 : a0 + (c1 - c0)],
                lhsT=kT_h[:, 128 * i : 128 * (i + 1)],
                rhs=qT_h[:, c_lo + c0 : c_lo + c1],
                start=True,
                stop=True,
            )
            nc.scalar.activation(
                pt[:, i, off + c0 : off + c1], ps[:, a0 : a0 + (c1 - c0)], EXP
            )
            st["act"] = max(st["act"], st["pe"]) + (c1 - c0) * ACT_NS + 160.0
        if 128 * i >= SLAB * s:  # diagonal block: zero the tq < tk half
            # DVE: lower latency than Pool's q7 launch on the exp->mask->PV chain
            eng = nc.gpsimd if os.environ.get("K_MASK_POOL") else nc.vector
            eng.tensor_mul(
                pt[:, i, off : off + 128], pt[:, i, off : off + 128], gemask
            )

    def emit_pv(h, s, jl, pt):
        # y[tq block, head h cols] = sum_tk P~ v, col 64 = rowsum
        u = s * HPC + h
        if pv_done[u][jl]:
            return False
        pv_done[u][jl] = True
        jg = (SLAB // 128) * s + jl
        for i in range(min(jg + 1, NT)):
            emit_v(i)
        pe(65.0 * (jg + 1) * PE_NS)
        ps = ps_y.tile([128, 68], F32, tag="y")
        for i in range(jg + 1):
            nc.tensor.matmul(
                ps[:, 0:65],
                lhsT=pt[:, i, 128 * jl : 128 * (jl + 1)],
                rhs=v_aug[:, i, 66 * h : 66 * h + 65],
                start=(i == 0),
                stop=(i == jg),
            )
        rinv = stage.tile([128, 1], F32, tag="rinv")
        nc.vector.reciprocal(rinv, ps[:, DH : DH + 1])
        nc.vector.tensor_scalar_mul(
            y_all[:, jg, DH * h : DH * (h + 1)], ps[:, 0:DH], rinv
        )
        return True

    def tp_trans(j):
        for dm in range(2):
            pst = ps_mm.tile([128, 128], BF16, tag="mm")
            nc.tensor.transpose(pst, y_all[:, j, 128 * dm : 128 * (dm + 1)], ident)
            nc.vector.tensor_copy(yT_s[:, dm, 128 * j : 128 * (j + 1)], pst)

    def tp_proj(j):
        for n in range(2):
            ps = ps_mm.tile([128, 512], F32, tag="mm")
            for dm in range(2):
                nc.tensor.matmul(
                    ps,
                    lhsT=yT_s[:, dm, 128 * j : 128 * (j + 1)],
                    rhs=wpr_s[:, dm, 512 * n : 512 * (n + 1)],
                    start=(dm == 0),
                    stop=(dm == 1),
                )
            nc.vector.tensor_copy(ost[:, j, 512 * n : 512 * (n + 1)], ps)

    def emit_tp_batch(js):
        # transposes of the batch first, then projections, so each j's
        # PE->Pool->PE chain overlaps the next j's PE work
        js = [j for j in js if not tp_done[j]]
        for j in js:
            tp_done[j] = True
            pe(256.0 * PE_NS, rdy_wpr)
            tp_trans(j)
        for j in js:
            pe(2048.0 * PE_NS)
            tp_proj(j)

    # ---- filler queues: PE work to interleave between S strips ----
    # qk/v entries: (dma_ready_estimate_ns, unit); pv entries: (strip_seq
    # at queue time, unit) — ready one full strip later; tp: FIFO
    qk_q = deque()
    for m, n in [(1, 1), (2, 0), (3, 0), (2, 1), (3, 1), (0, 2), (1, 2), (0, 3),
                 (1, 3), (2, 2), (3, 2), (2, 3), (3, 3)]:
        qk_q.append((max(rdy_wqkA if m < 2 else rdy_wqkB, rdy_cb[n]), ("qk", m, n)))
    v_q = deque()
    for j in range(NT):
        v_q.append((max(rdy_wv, rdy_cb[j // 4]), ("v", j)))
    pv_q = deque()
    tp_q = deque()
    strip_seq = [0]  # strips emitted so far

    pt_tiles = {}
    pv_count = [0] * NT  # heads completed per tq block

    def run_unit(u):
        kind = u[0]
        if kind == "qk":
            emit_qk(u[1], u[2])
        elif kind == "v":
            emit_v(u[1])
        elif kind == "pv":
            _, h, s, jl = u
            if emit_pv(h, s, jl, pt_tiles[(h, s)]):
                jg = (SLAB // 128) * s + jl
                pv_count[jg] += 1
                if pv_count[jg] == HPC and phase >= 5:
                    tp_q.append(("tp", jg))
        elif kind == "tp":
            js = [u[1]]
            if tp_q and len(js) < 2:
                js.append(tp_q.popleft()[1])
            emit_tp_batch(js)

    def pick(force=False):
        # priority: pv (frees pt/psum, feeds tp) > qk > v > tp
        if pv_q and pv_q[0][0] < strip_seq[0]:
            return pv_q.popleft()[1]
        for q in (qk_q, v_q):
            if q and q[0][0] <= st["pe"]:
                return q.popleft()[1]
        if tp_q:
            return tp_q.popleft()
        if force:
            for q in (pv_q, qk_q, v_q):
                if q:
                    return q.popleft()[1]
        return None

    def pace(next_exp_cost):
        # fill only to keep PE from stalling on the ps_s ring (2 outstanding
        # exps): pop while PE's frontier would reach the next strip before
        # ACT has drained all but ~1.5 exps of its backlog
        while st["pe"] < st["act"] - 1.5 * next_exp_cost:
            u = pick()
            if u is None:
                break
            run_unit(u)

    def flush_pv(h, s):
        u = s * HPC + h
        for jl in range(SLAB // 128):
            if not pv_done[u][jl]:
                run_unit(("pv", h, s, jl))

    # ---- main rotation: slab-major, heads round-robin ----
    if phase == 2:
        for m in range(4):
            for n in range(4):
                emit_qk(m, n)
        for j in range(NT):
            emit_v(j)
        return
    emit_qk(0, 0)
    emit_qk(1, 0)
    rotation = [(h, s) for s in range(NS) for h in range(HPC)]
    for idx, (h, s) in enumerate(rotation):
        if idx >= 2:
            ph, ps_ = rotation[idx - 2]
            flush_pv(ph, ps_)  # frees that head's pt buffer (pool bufs=2)
        pt = pt_pool.tile([128, NT, SLAB], BF16, tag="pt")
        pt_tiles[(h, s)] = pt
        ntk = (SLAB // 128) * (s + 1)
        for i in range(ntk):
            s_strip(h, s, i, pt, split=(idx == 0 and i < 4))
            strip_seq[0] += 1
            if 128 * i >= SLAB * s and phase >= 4:
                # this strip's exp completes tq block i: queue its PV
                pv_q.append(
                    (strip_seq[0], ("pv", h, s, i - (SLAB // 128) * s))
                )
            nxt = SLAB * (s + 1) - max(SLAB * s, 128 * (i + 1))
            pace(nxt * ACT_NS + 160.0)
    if phase >= 4:
        flush_pv(*rotation[-2])
        flush_pv(*rotation[-1])
    while True:
        u = pick(force=True)
        if u is None:
            break
        run_unit(u)
    if phase >= 5:
        emit_tp_batch(list(range(NT)))

    if not pipelined_out:
        # single-shot build: ship this iteration's output at the end
        _emit_out_dmas(nc, out_d, ost)
    return ost


def _get_nc():
    if "nc" not in _NC_CACHE:
        _NC_CACHE["nc"] = _build_program()
    return _NC_CACHE["nc"]


def make_in_maps(x, w_qkv, w_proj):
    bf16 = ml_dtypes.bfloat16
    scale = np.float32(DH**-0.25)
    x = np.asarray(x, dtype=np.float32)
    w_qkv = np.asarray(w_qkv, dtype=np.float32)
    w_proj = np.asarray(w_proj, dtype=np.float32)
    xT_b = [np.ascontiguousarray(x[b].T).astype(bf16) for b in range(B)]
    in_maps = []
    for c in range(NCORES):
        b, g = c // HPC, c % HPC
        cs = slice(g * HPC * DH, (g + 1) * HPC * DH)  # 256 cols of this head group
        wq = w_qkv[:, 0 * D : 1 * D][:, cs] * scale
        wk = w_qkv[:, 1 * D : 2 * D][:, cs] * scale
        # column order [q01, k01, q23, k23] so the first 256-col DMA chunk
        # carries everything heads 0,1 need
        wqk = np.concatenate(
            [wq[:, 0:128], wk[:, 0:128], wq[:, 128:256], wk[:, 128:256]], axis=1
        )
        in_maps.append(
            {
                "xT": xT_b[b],
                "wqk": np.ascontiguousarray(wqk).astype(bf16),
                "wv": np.ascontiguousarray(w_qkv[:, 2 * D : 3 * D][:, cs]).astype(bf16),
                "wpr": np.ascontiguousarray(w_proj[cs, :]).astype(bf16),
            }
        )
    return in_maps


def kernel(x, w_qkv, w_proj):
    global LAST_RESULTS
    nc = _get_nc()
    in_maps = make_in_maps(x, w_qkv, w_proj)
    res = run_bass_kernel_spmd(nc, in_maps, list(range(NCORES)), trace=TRACE)
    LAST_RESULTS = res
    parts = [np.asarray(res.results[c]["out"], dtype=np.float32) for c in range(NCORES)]
    out = np.stack([sum(parts[b * HPC : (b + 1) * HPC]) for b in range(B)], axis=0)
    return out.astype(np.float32)


# revision 37
# speedup vs baseline: 1.2928x; 1.2928x over previous
"""Causal self-attention (B=2, T=2048, D=1024, 16 heads) on 8 trn2 cores.

Sharding: data-parallel over batch (4 cores per batch element), tensor-parallel
over heads (4 heads per core). Each core computes qkv/attention/proj for its
4 heads and produces a partial [T, D] projection output; the host sums the 4
partials of each batch element.

Schedule: the kernel is a software pipeline paced around the Activation
engine (exp is the second-largest engine load after the PE matmuls). Heads
rotate at (head, tq-slab) granularity: S^T strips stream onto PE and exp onto
ACT, while qkv projection / v / PV / output-projection matmuls are emitted as
PE filler between strips so the PE never waits on ACT's exp backlog. Input and
output DMAs are issued from SP; in the For_i timing build, UNROLL bodies are
emitted per loop iteration (For_i has an all-engine barrier per iteration)
with the output DMA software-pipelined one body behind its compute.
"""

from collections import deque

import numpy as np
import ml_dtypes

import concourse.mybir as mybir
import concourse.tile as tile
from concourse import bacc
from concourse.bass_utils import run_bass_kernel_spmd
from concourse.masks import make_identity, make_upper_triangular

B, T, D = 2, 2048, 1024
NH, DH = 16, 64
HPC = 4  # heads per core
NCORES = 8
KT = D // 128  # 8 contraction chunks for qkv matmuls
NT = T // 128  # 16 sequence chunks
SLAB = 1024  # tq columns per attention slab
NS = T // SLAB
import os
UNROLL = int(os.environ.get("K_UNROLL", "2"))  # bodies per For_i iteration

BF16 = mybir.dt.bfloat16
F16 = mybir.dt.float16
F32 = mybir.dt.float32
EXP = mybir.ActivationFunctionType.Exp

PE_NS = 1.0 / 2.4  # ns per PE cycle (full pstate)
ACT_NS = 1.0 / 1.2

TRACE = False
LAST_RESULTS = None
_NC_CACHE = {}


def _patch_ldw_opt():
    # walrus ships with the ldweights-optimization pass disabled; flipping it
    # lets codegen overlap/dedup PE weight loads (measured ~40-50ns each,
    # serialized, across ~1k matmuls here)
    import concourse.bass_utils as _bu

    if getattr(_bu, "_ldw_patched", False):
        return
    _orig = _bu.run_command

    def _rc(cmd, *a, **k):
        if isinstance(cmd, list):
            cmd = [
                "--enable-ldw-opt=true" if c == "--enable-ldw-opt=false" else c
                for c in cmd
            ]
        return _orig(cmd, *a, **k)

    _bu.run_command = _rc
    _bu._ldw_patched = True


if os.environ.get("K_LDWOPT"):
    _patch_ldw_opt()


def _build_program(loop_n=None, unroll=None, phase=5):
    nc = bacc.Bacc("TRN2", target_bir_lowering=False, debug=False, num_devices=NCORES)
    xT_d = nc.dram_tensor("xT", [D, T], BF16, kind="ExternalInput").ap()
    # wqk columns reordered host-side to [q01, k01, q23, k23] (128 each)
    wqk_d = nc.dram_tensor("wqk", [D, 2 * HPC * DH], BF16, kind="ExternalInput").ap()
    wv_d = nc.dram_tensor("wv", [D, HPC * DH], BF16, kind="ExternalInput").ap()
    wpr_d = nc.dram_tensor("wpr", [HPC * DH, D], BF16, kind="ExternalInput").ap()
    out_d = nc.dram_tensor("out", [T, D], F16, kind="ExternalOutput").ap()

    with tile.TileContext(nc) as tc:
        with (
            tc.tile_pool(name="setup", bufs=1) as setup,
            tc.tile_pool(name="big", bufs=1) as big,
            tc.tile_pool(name="pt_pool", bufs=2) as pt_pool,
            tc.tile_pool(name="stage", bufs=2) as stage,
            tc.tile_pool(name="ps_mm", bufs=int(os.environ.get("K_MMBUFS", "2")), space="PSUM") as ps_mm,
            tc.tile_pool(name="ps_s", bufs=int(os.environ.get("K_SBUFS", "2")), space="PSUM") as ps_s,
            tc.tile_pool(name="ps_y", bufs=2, space="PSUM") as ps_y,
        ):
            # ---- one-time setup (outside the For_i loop) ----
            ident = setup.tile([128, 128], BF16)
            make_identity(nc, ident)
            # exp table load happens here, off the steady-state critical path
            warm = setup.tile([128, 1], F32)
            nc.vector.memset(warm, 0.0)
            nc.scalar.activation(warm, warm, EXP)
            # gemask[p, f] = 1.0 where f >= p: valid (tq >= tk) part of a
            # diagonal 128x128 block of S^T.
            gemask = setup.tile([128, 128], BF16)
            make_upper_triangular(nc, gemask, val=1.0, diag=True)
            # v in natural [tk, d] layout + a ones-column per head for rowsums;
            # only the ones columns need initialization (col 64 of each 66).
            v_aug = setup.tile([128, NT, 66 * HPC], BF16)
            # persistent output staging: written by each body's copies, read
            # by the next body's (or the drain's) output DMAs
            ost = setup.tile([128, NT, D], F16)
            ones_cols = v_aug.rearrange("p n (h c) -> p n h c", c=66)[
                :, :, :, DH : DH + 1
            ]
            nc.vector.memset(ones_cols, 1.0)

            pools = (big, pt_pool, stage, ps_mm, ps_s, ps_y)
            consts = (ident, gemask, v_aug, ost)
            if unroll is not None:
                # straight-line repetition of the loop body for TimelineSim
                # steady-state analysis (For_i register loops aren't simulable)
                for _ in range(unroll):
                    _emit(nc, tc, pools, consts, xT_d, wqk_d, wv_d,
                          wpr_d, out_d, pipelined_out=(phase >= 5),
                          phase=phase)
                if phase >= 5:
                    _emit_out_dmas(nc, out_d, ost)
            elif loop_n is None:
                _emit(nc, tc, pools, consts, xT_d, wqk_d, wv_d, wpr_d, out_d,
                      pipelined_out=False)
            else:
                # For_i inserts an all-engine barrier per iteration, which
                # serializes consecutive bodies. Unroll UNROLL bodies per
                # iteration so they overlap (input DMAs / output DMAs / tail
                # of one body run under compute of the next).
                assert loop_n % UNROLL == 0, (loop_n, UNROLL)
                hints = (
                    mybir.EngineType.PE,
                    mybir.EngineType.Activation,
                    mybir.EngineType.DVE,
                    mybir.EngineType.SP,
                    mybir.EngineType.Pool,
                )
                with tc.For_i(0, loop_n // UNROLL, 1, hint_engines=hints,
                              staggered_reset=bool(os.environ.get("K_STAGGER"))):
                    for _ in range(UNROLL):
                        _emit(nc, tc, pools, consts, xT_d, wqk_d, wv_d,
                              wpr_d, out_d, pipelined_out=(phase >= 5),
                              phase=phase)
                # drain: the loop body DMAs the *previous* iteration's output,
                # so the final iteration's staged output goes out here
                if phase >= 5:
                    _emit_out_dmas(nc, out_d, ost)
    nc.compile()
    return nc


def _emit_out_dmas(nc, out_d, ost):
    for g in range(4):
        nc.sync.dma_start(
            out=out_d[512 * g : 512 * (g + 1), :].rearrange(
                "(j p) c -> p j c", p=128
            ),
            in_=ost[:, 4 * g : 4 * (g + 1), :],
        )


def _emit(nc, tc, pools, consts, xT_d, wqk_d, wv_d, wpr_d, out_d,
          pipelined_out=False, phase=5):
    # phase: 1=DMA only, 2=+qk/v, 3=+S/exp/mask, 4=+PV, 5=full
    big, pt_pool, stage, ps_mm, ps_s, ps_y = pools
    ident, gemask, v_aug, ost = consts

    xT_s = big.tile([128, KT, T], BF16, tag="xT")
    wqk_s = big.tile([128, KT, 2 * HPC * DH], BF16, tag="wqk")
    wv_s = big.tile([128, KT, HPC * DH], BF16, tag="wv")
    wpr_s = big.tile([128, 2, D], BF16, tag="wpr")
    # q^T / k^T in [d, T] layout: jt holds heads 2*jt (parts 0:64) and
    # 2*jt+1 (parts 64:128).
    qT_s = big.tile([128, 2, T], BF16, tag="qT")
    kT_s = big.tile([128, 2, T], BF16, tag="kT")
    y_all = big.tile([128, NT, HPC * DH], BF16, tag="y")
    yT_s = big.tile([128, 2, T], BF16, tag="yT")

    # ---- input DMAs, SP only; ordered for earliest first matmul ----
    xT_r = xT_d.rearrange("(a p) t -> p a t", p=128)
    wqk_r = wqk_d.rearrange("(a p) n -> p a n", p=128)
    nc.sync.dma_start(out=wqk_s[:, :, 0:256], in_=wqk_r[:, :, 0:256])
    nc.sync.dma_start(out=xT_s[:, :, 0:512], in_=xT_r[:, :, 0:512])
    nc.sync.dma_start(out=xT_s[:, :, 512:1024], in_=xT_r[:, :, 512:1024])
    nc.sync.dma_start(out=wqk_s[:, :, 256:512], in_=wqk_r[:, :, 256:512])
    nc.sync.dma_start(out=xT_s[:, :, 1024:1536], in_=xT_r[:, :, 1024:1536])
    nc.sync.dma_start(out=xT_s[:, :, 1536:2048], in_=xT_r[:, :, 1536:2048])
    nc.sync.dma_start(out=wv_s, in_=wv_d.rearrange("(a p) n -> p a n", p=128))
    nc.sync.dma_start(out=wpr_s, in_=wpr_d.rearrange("(a p) n -> p a n", p=128))

    if pipelined_out:
        # software-pipelined output: DMA the PREVIOUS body's staged output
        # (its copies finished by the end of that body), issued from ACT
        # after this body's input DMAs so inputs win the DMA engines first.
        # Body 0 ships garbage that later bodies' DMAs overwrite; the
        # post-loop drain ships the final body's data.
        _emit_out_dmas(nc, out_d, ost)
    # virtual completion estimates (ns) for pacing only
    rdy_wqkA, rdy_cb = 1500.0, [4400.0, 7300.0, 11700.0, 14600.0]
    rdy_wqkB, rdy_wv, rdy_wpr = 8800.0, 16100.0, 17700.0
    if phase == 1:
        return

    # ---- emission state ----
    st = {"pe": 0.0, "act": 0.0}
    qk_done = [[False] * 4 for _ in range(4)]  # [m'][n]; m' 0=q01 1=k01 2=q23 3=k23
    v_done = [False] * NT
    pv_done = [[False] * (SLAB // 128) for _ in range(2 * HPC)]  # [slab*4+h][jl]
    tp_done = [False] * NT

    def pe(cost, ready=0.0):
        st["pe"] = max(st["pe"], ready) + cost

    def emit_qk(m, n):
        # q^T or k^T block: wqk m'-block, T cols [512n, 512n+512)
        if qk_done[m][n]:
            return
        qk_done[m][n] = True
        pe(1707.0, max(rdy_wqkA if m < 2 else rdy_wqkB, rdy_cb[n]))
        ps = ps_mm.tile([128, 512], F32, tag="mm")
        for t in range(KT):
            nc.tensor.matmul(
                ps,
                lhsT=wqk_s[:, t, 128 * m : 128 * (m + 1)],
                rhs=xT_s[:, t, 512 * n : 512 * (n + 1)],
                start=(t == 0),
                stop=(t == KT - 1),
            )
        dst = qT_s if m % 2 == 0 else kT_s
        nc.vector.tensor_copy(dst[:, m // 2, 512 * n : 512 * (n + 1)], ps)

    def emit_v(j):
        # v rows [128j, 128j+128) = x @ wv, scattered into v_aug
        if v_done[j]:
            return
        v_done[j] = True
        pe(853.0, max(rdy_wv, rdy_cb[j // 4]))
        ps = ps_mm.tile([128, HPC * DH], F32, tag="mm")
        for t in range(KT):
            nc.tensor.matmul(
                ps,
                lhsT=xT_s[:, t, 128 * j : 128 * (j + 1)],
                rhs=wv_s[:, t, :],
                start=(t == 0),
                stop=(t == KT - 1),
            )
        nc.vector.tensor_copy(
            v_aug[:, j, :].rearrange("p (h c) -> p h c", c=66)[:, :, 0:DH],
            ps.rearrange("p (h c) -> p h c", c=DH),
        )

    def s_strip(h, s, i, pt, split=False):
        # S^T strip: tk block i vs tq cols [c_lo, SLAB*(s+1)) of slab s,
        # exp'd into pt[:, i, :]. Diagonal block gets gemask on DVE.
        # split=True emits exp per 512-col chunk (earlier ACT start; the
        # second chunk's q columns may still be in flight on DMA).
        jt, base = h // 2, 64 * (h % 2)
        qm, km = 2 * jt, 2 * jt + 1
        emit_qk(qm, 2 * s)
        if not split:
            emit_qk(qm, 2 * s + 1)
        emit_qk(km, i // 4)
        qT_h = qT_s[base : base + 64, jt, :]
        kT_h = kT_s[base : base + 64, jt, :]
        c_lo = max(SLAB * s, 128 * i)
        w = SLAB * (s + 1) - c_lo
        off = c_lo - SLAB * s
        ps = ps_s.tile([128, SLAB], F32, tag="s")
        if split:
            # chunk at global 512-col boundaries so exp can fire per chunk
            chunks, b = [], 0
            while b < w:
                e = min(w, b + 512 - (c_lo + b) % 512)
                chunks.append((b, e))
                b = e
        else:
            chunks = [(c, min(c + 512, w)) for c in range(0, w, 512)]
        # split chunks sit at slab-aligned ps offsets so each matmul output
        # stays within one 512-col PSUM bank
        pso = off if split else 0
        for ci, (c0, c1) in enumerate(chunks):
            if split and ci > 0:
                emit_qk(qm, (c_lo + c0) // 512)
            pe((c1 - c0) * PE_NS)
            nc.tensor.matmul(
                ps[:, pso + c0 : pso + c1],
                lhsT=kT_h[:, 128 * i : 128 * (i + 1)],
                rhs=qT_h[:, c_lo + c0 : c_lo + c1],
                start=True,
                stop=True,
            )
            if split:
                nc.scalar.activation(
                    pt[:, i, off + c0 : off + c1], ps[:, pso + c0 : pso + c1], EXP
                )
                st["act"] = max(st["act"], st["pe"]) + (c1 - c0) * ACT_NS + 160.0
        if not split:
            nc.scalar.activation(pt[:, i, off : off + w], ps[:, 0:w], EXP)
            st["act"] = max(st["act"], st["pe"]) + w * ACT_NS + 160.0
        if 128 * i >= SLAB * s:  # diagonal block: zero the tq < tk half
            # DVE: lower latency than Pool's q7 launch on the exp->mask->PV chain
            eng = nc.gpsimd if os.environ.get("K_MASK_POOL") else nc.vector
            eng.tensor_mul(
                pt[:, i, off : off + 128], pt[:, i, off : off + 128], gemask
            )

    def emit_pv(h, s, jl, pt):
        # y[tq block, head h cols] = sum_tk P~ v, col 64 = rowsum
        u = s * HPC + h
        if pv_done[u][jl]:
            return False
        pv_done[u][jl] = True
        jg = (SLAB // 128) * s + jl
        for i in range(min(jg + 1, NT)):
            emit_v(i)
        pe(65.0 * (jg + 1) * PE_NS)
        ps = ps_y.tile([128, 68], F32, tag="y")
        for i in range(jg + 1):
            nc.tensor.matmul(
                ps[:, 0:65],
                lhsT=pt[:, i, 128 * jl : 128 * (jl + 1)],
                rhs=v_aug[:, i, 66 * h : 66 * h + 65],
                start=(i == 0),
                stop=(i == jg),
            )
        rinv = stage.tile([128, 1], F32, tag="rinv")
        nc.vector.reciprocal(rinv, ps[:, DH : DH + 1])
        nc.vector.tensor_scalar_mul(
            y_all[:, jg, DH * h : DH * (h + 1)], ps[:, 0:DH], rinv
        )
        return True

    def tp_trans(j):
        for dm in range(2):
            pst = ps_mm.tile([128, 128], BF16, tag="mm")
            nc.tensor.transpose(pst, y_all[:, j, 128 * dm : 128 * (dm + 1)], ident)
            nc.vector.tensor_copy(yT_s[:, dm, 128 * j : 128 * (j + 1)], pst)

    def tp_proj(j):
        for n in range(2):
            ps = ps_mm.tile([128, 512], F32, tag="mm")
            for dm in range(2):
                nc.tensor.matmul(
                    ps,
                    lhsT=yT_s[:, dm, 128 * j : 128 * (j + 1)],
                    rhs=wpr_s[:, dm, 512 * n : 512 * (n + 1)],
                    start=(dm == 0),
                    stop=(dm == 1),
                )
            nc.vector.tensor_copy(ost[:, j, 512 * n : 512 * (n + 1)], ps)

    def emit_tp_batch(js):
        # transposes of the batch first, then projections, so each j's
        # PE->Pool->PE chain overlaps the next j's PE work
        js = [j for j in js if not tp_done[j]]
        for j in js:
            tp_done[j] = True
            pe(256.0 * PE_NS, rdy_wpr)
            tp_trans(j)
        for j in js:
            pe(2048.0 * PE_NS)
            tp_proj(j)

    # ---- filler queues: PE work to interleave between S strips ----
    # qk/v entries: (dma_ready_estimate_ns, unit); pv entries: (strip_seq
    # at queue time, unit) — ready one full strip later; tp: FIFO
    qk_q = deque()
    for m, n in [(1, 1), (2, 0), (3, 0), (2, 1), (3, 1), (0, 2), (1, 2), (0, 3),
                 (1, 3), (2, 2), (3, 2), (2, 3), (3, 3)]:
        qk_q.append((max(rdy_wqkA if m < 2 else rdy_wqkB, rdy_cb[n]), ("qk", m, n)))
    v_q = deque()
    for j in range(NT):
        v_q.append((max(rdy_wv, rdy_cb[j // 4]), ("v", j)))
    pv_q = deque()
    tp_q = deque()
    strip_seq = [0]  # strips emitted so far

    pt_tiles = {}
    pv_count = [0] * NT  # heads completed per tq block

    def run_unit(u):
        kind = u[0]
        if kind == "qk":
            emit_qk(u[1], u[2])
        elif kind == "v":
            emit_v(u[1])
        elif kind == "pv":
            _, h, s, jl = u
            if emit_pv(h, s, jl, pt_tiles[(h, s)]):
                jg = (SLAB // 128) * s + jl
                pv_count[jg] += 1
                if pv_count[jg] == HPC and phase >= 5:
                    tp_q.append(("tp", jg))
        elif kind == "tp":
            js = [u[1]]
            if tp_q and len(js) < 2:
                js.append(tp_q.popleft()[1])
            emit_tp_batch(js)

    def pick(force=False):
        # priority: pv (frees pt/psum, feeds tp) > qk > v > tp
        if pv_q and pv_q[0][0] < strip_seq[0]:
            return pv_q.popleft()[1]
        for q in (qk_q, v_q):
            if q and q[0][0] <= st["pe"]:
                return q.popleft()[1]
        if tp_q:
            return tp_q.popleft()
        if force:
            for q in (pv_q, qk_q, v_q):
                if q:
                    return q.popleft()[1]
        return None

    def pace(next_exp_cost):
        # fill only to keep PE from stalling on the ps_s ring (2 outstanding
        # exps): pop while PE's frontier would reach the next strip before
        # ACT has drained all but ~1.5 exps of its backlog
        while st["pe"] < st["act"] - 1.5 * next_exp_cost:
            u = pick()
            if u is None:
                break
            run_unit(u)

    def flush_pv(h, s):
        u = s * HPC + h
        for jl in range(SLAB // 128):
            if not pv_done[u][jl]:
                run_unit(("pv", h, s, jl))

    # ---- main rotation: slab-major, heads round-robin ----
    if phase == 2:
        for m in range(4):
            for n in range(4):
                emit_qk(m, n)
        for j in range(NT):
            emit_v(j)
        return
    emit_qk(0, 0)
    emit_qk(1, 0)
    rotation = [(h, s) for s in range(NS) for h in range(HPC)]
    for idx, (h, s) in enumerate(rotation):
        if idx >= 2:
            ph, ps_ = rotation[idx - 2]
            flush_pv(ph, ps_)  # frees that head's pt buffer (pool bufs=2)
        pt = pt_pool.tile([128, NT, SLAB], BF16, tag="pt")
        pt_tiles[(h, s)] = pt
        ntk = (SLAB // 128) * (s + 1)
        for i in range(ntk):
            s_strip(h, s, i, pt, split=(idx == 0 and i < 4))
            strip_seq[0] += 1
            if 128 * i >= SLAB * s and phase >= 4:
                # this strip's exp completes tq block i: queue its PV
                pv_q.append(
                    (strip_seq[0], ("pv", h, s, i - (SLAB // 128) * s))
                )
            nxt = SLAB * (s + 1) - max(SLAB * s, 128 * (i + 1))
            pace(nxt * ACT_NS + 160.0)
    if phase >= 4:
        flush_pv(*rotation[-2])
        flush_pv(*rotation[-1])
    while True:
        u = pick(force=True)
        if u is None:
            break
        run_unit(u)
    if phase >= 5:
        emit_tp_batch(list(range(NT)))

    if not pipelined_out:
        # single-shot build: ship this iteration's output at the end
        _emit_out_dmas(nc, out_d, ost)
    return ost


def _get_nc():
    if "nc" not in _NC_CACHE:
        _NC_CACHE["nc"] = _build_program()
    return _NC_CACHE["nc"]


def make_in_maps(x, w_qkv, w_proj):
    bf16 = ml_dtypes.bfloat16
    scale = np.float32(DH**-0.25)
    x = np.asarray(x, dtype=np.float32)
    w_qkv = np.asarray(w_qkv, dtype=np.float32)
    w_proj = np.asarray(w_proj, dtype=np.float32)
    xT_b = [np.ascontiguousarray(x[b].T).astype(bf16) for b in range(B)]
    in_maps = []
    for c in range(NCORES):
        b, g = c // HPC, c % HPC
        cs = slice(g * HPC * DH, (g + 1) * HPC * DH)  # 256 cols of this head group
        wq = w_qkv[:, 0 * D : 1 * D][:, cs] * scale
        wk = w_qkv[:, 1 * D : 2 * D][:, cs] * scale
        # column order [q01, k01, q23, k23] so the first 256-col DMA chunk
        # carries everything heads 0,1 need
        wqk = np.concatenate(
            [wq[:, 0:128], wk[:, 0:128], wq[:, 128:256], wk[:, 128:256]], axis=1
        )
        in_maps.append(
            {
                "xT": xT_b[b],
                "wqk": np.ascontiguousarray(wqk).astype(bf16),
                "wv": np.ascontiguousarray(w_qkv[:, 2 * D : 3 * D][:, cs]).astype(bf16),
                "wpr": np.ascontiguousarray(w_proj[cs, :]).astype(bf16),
            }
        )
    return in_maps


def kernel(x, w_qkv, w_proj):
    global LAST_RESULTS
    nc = _get_nc()
    in_maps = make_in_maps(x, w_qkv, w_proj)
    res = run_bass_kernel_spmd(nc, in_maps, list(range(NCORES)), trace=TRACE)
    LAST_RESULTS = res
    parts = [np.asarray(res.results[c]["out"], dtype=np.float32) for c in range(NCORES)]
    out = np.stack([sum(parts[b * HPC : (b + 1) * HPC]) for b in range(B)], axis=0)
    return out.astype(np.float32)
